# revision 15
# baseline (speedup 1.0000x reference)
"""GAT layer (B=4, N=2048, D=256, H=4) on 8 trn2 NeuronCores.

Sharding: core c -> (b = c//2, i-half = c%2).  Each core computes
out[b, ihalf*1024:(ihalf+1)*1024, :]; h is computed on-device from the full
x[b] (passed pre-transposed as x[b].T, column-permuted so the core's own
i-half comes first).

Math: with z = s_src[i] + s_dst[j], the reference computes
    alpha = softmax_j(mask(leaky_relu(z)));  out = alpha @ h_head.
Softmax is shift-invariant per destination row i, so we use shifted
unnormalized weights (exact same alpha).  F1 = exp(s_dst) is folded into
the attention weights (not into h):
    P[j,i] = adj[j,i] * F1[j] * max(E2[i]*G[j], 1)
           = adj[j,i] * max(E2[i]*GF1[j], F1[j])
with GF1 = exp(0.2*s_dst), E2 = exp(-0.8*s_src)
(using exp(max(a,b)) = max(exp a, exp b) and leaky = max(z, 0.2 z)).
Row sums come from an appended ones-column in the aggregation matmul
stationary: psoT = [h_head | 1].T @ P^T; numerator and denominator are
DMA'd out raw and the final divide + transpose happens on the host.

The per-(head, j-tile) elementwise work P^T is split across three engines:
  D  tiles: DVE tensor_scalar (max(e2rep*gf1, f1)) + tensor_tensor (*adjT)
  S1 tiles: scalar ACT t=Relu(gf1*e2rep - f1); DVE STT pt=(t+f1)*adjT
  S2 tiles: scalar ACT as above; gpsimd STT
"""

import sys

for _p in ("/opt/trn_rl_repo", "/root/.axon_site/_ro/trn_rl_repo"):
    if _p not in sys.path:
        sys.path.insert(0, _p)

import ml_dtypes
import numpy as np

import concourse.bass as bass
import concourse.mybir as mybir
from concourse import tile
from concourse.bass_utils import run_bass_kernel_spmd
from concourse.vector_clock import ScopedClock

F32 = mybir.dt.float32
F16 = mybir.dt.float16
BF16 = mybir.dt.bfloat16
AF = mybir.ActivationFunctionType
ALU = mybir.AluOpType

B, N, D, H, HD = 4, 2048, 256, 4, 64
NEG_SLOPE = 0.2
P = 128
NI = N // 2          # i-rows per core (1024)
NT = N // P          # 16 j tiles
KT = D // P          # 2 k tiles
NCORES = 8
WC = D + 2 * H       # 264 aug cols: [W.T | Wt@A_src | Wt@A_dst]
HP = H * (HD + 1)    # 260 hplus cols per j-tile

# per-(head, j-tile) elementwise engine assignment
#   0 = DVE TS + DVE TT; 1 = scalar ACT + DVE STT; 3 = gpsimd TS + DVE TT
PAT0 = [0, 3, 0, 3, 0, 3, 3, 0, 3, 1, 3, 0, 3, 3, 0, 1]
PAT1 = [3, 1, 0, 1, 3, 3, 1, 3, 0, 3, 1, 3, 3, 1, 3, 1]
PATHS = [PAT0, PAT1, PAT1, PAT1]


def _patch_tile_drain():
    """walrus rejects >1 sem wait on one instruction in this toolchain; split
    the TileContext tail drain's waits across consecutive SP drains."""
    if getattr(tile.TileContext, "_drain_patched", False):
        return

    def _drain_and_barrier(self, tick_clock, wait_clock):
        nc = self.nc
        drain_inst = nc.sync.drain()
        wait_clock.add_sem_waits(
            drain_inst.ins, ScopedClock({None: tick_clock.global_clock})
        )
        si = drain_inst.ins.sync_info
        waits = list(si.on_wait) if (si and si.on_wait) else []
        if len(waits) > 1:
            ups = list(si.on_update) if (si and si.on_update) else []
            drain_inst.ins.sync_info = mybir.SyncInfo(on_wait=waits[:1], on_update=ups)
            for i in range(1, len(waits)):
                extra = nc.sync.drain()
                extra.ins.sync_info = mybir.SyncInfo(
                    on_wait=waits[i : i + 1], on_update=[]
                )
        nc.all_engine_barrier()
        assert self.sems is not None
        popped = nc._tile_sem_poison_stack.pop()
        assert popped is self._sem_poison
        nc.clear_and_free_semaphores(list(self.sems.allocated().values()))
        nc.all_engine_barrier()

    tile.TileContext._drain_and_barrier = _drain_and_barrier
    tile.TileContext._drain_patched = True


def _split_waits(nc, maxw=1):
    """Hoist excess sem waits onto same-engine EventSemaphore carriers placed
    just before the instruction (same engine + program order => equivalent)."""
    n_split = 0
    for f in nc.m.functions:
        for bb in f.blocks:
            insts = list(bb.instructions)
            out = []
            changed = False
            for inst in insts:
                si = inst.sync_info
                waits = list(si.on_wait) if (si and si.on_wait) else []
                if len(waits) > maxw and inst.engine is not None:
                    changed = True
                    extra, keep = waits[:-maxw], waits[-maxw:]
                    for k in range(0, len(extra), maxw):
                        d = mybir.InstEventSemaphore(
                            name=f"{inst.name}-wsplit{k}", ins=[], outs=[]
                        )
                        d.engine = inst.engine
                        d.sync_info = mybir.SyncInfo(
                            on_wait=extra[k : k + maxw], on_update=[]
                        )
                        out.append(d)
                        n_split += 1
                    ups = list(si.on_update) if (si and si.on_update) else []
                    inst.sync_info = mybir.SyncInfo(on_wait=keep, on_update=ups)
                out.append(inst)
            if changed:
                bb.instructions = out
    return n_split


def build_nc(split_waits=True):
    _patch_tile_drain()
    nc = bass.Bass("TRN2", target_bir_lowering=False, debug=False)

    xbt = nc.dram_tensor("xbt", [D, N], F16, kind="ExternalInput")    # x[b].T perm
    wta = nc.dram_tensor("wta", [D, WC], F16, kind="ExternalInput")
    adjtb = nc.dram_tensor("adjtb", [N, NI], BF16, kind="ExternalInput")
    selm = nc.dram_tensor("selm", [H, H * P], BF16, kind="ExternalInput")
    outs = nc.dram_tensor("outs", [2 * H, HD + 1, 512], F32, kind="ExternalOutput")

    with tile.TileContext(nc) as tc:
        with (
            tc.tile_pool(name="const", bufs=1) as constp,
            tc.tile_pool(name="big", bufs=1) as bigp,
            tc.tile_pool(name="rows", bufs=1) as rowsp,
            tc.tile_pool(name="vwork", bufs=8) as vp,
            tc.tile_pool(name="twork", bufs=6) as tp,
            tc.tile_pool(name="ptwork", bufs=22) as ptp,
            tc.tile_pool(name="ostage", bufs=3) as ostagep,
            tc.tile_pool(name="psall", bufs=1, space="PSUM") as psall,
        ):
            def ps_tile(shape, name, bank):
                return psall.tile(shape, F32, tag=f"bank{bank}", name=name)

            pe_prev = [None]

            def pe(bi):
                # pin PE stream order: PSUM accumulation groups must stay
                # contiguous on PE (interleaving corrupts accumulation on HW)
                if pe_prev[0] is not None:
                    tile.add_dep_helper(bi.ins, pe_prev[0], reason="pe-order")
                pe_prev[0] = bi.ins
                return bi

            # ---- constants ----
            wta_sb = [
                constp.tile([P, WC], F16, tag=f"wta{kt}", name=f"wta_sb{kt}")
                for kt in range(KT)
            ]
            sel_sb = constp.tile([H, H * P], BF16, tag="selm")
            nc.sync.dma_start(sel_sb[:], selm[:])
            sels = [sel_sb[:, h * P : (h + 1) * P] for h in range(H)]
            for kt in range(KT):
                nc.sync.dma_start(wta_sb[kt][:], wta[kt * P : (kt + 1) * P, :])
            wta_r = [wta_sb[kt][:] for kt in range(KT)]

            # ---- big SBUF tensors ----
            xt_raw = bigp.tile([P, KT * N], F16, tag="xtraw")
            xt_r = xt_raw[:]
            adjt_all = bigp.tile([P, NT * NI], BF16, tag="adjt")
            e2rep = bigp.tile([P, H * NI], BF16, tag="e2rep")
            hplus = bigp.tile([P, NT * HP], BF16, tag="hplus")
            f1_sb = bigp.tile([P, NT * H], F32, tag="f1")
            gf1_sb = bigp.tile([P, NT * H], F32, tag="gf1")
            nf1_sb = bigp.tile([P, NT * H], F32, tag="nf1")
            er4 = rowsp.tile([H, NI], BF16, tag="er4")

            nc.gpsimd.memset(hplus[:], 1.0)

            # ---- input DMAs (batched, own-half x first) ----
            xbt3 = xbt[:].rearrange("(k p) n -> p k n", p=P)
            xt3 = xt_raw[:].rearrange("p (k n) -> p k n", k=KT)
            adj3_in = adjtb[:].rearrange("(t p) i -> p t i", p=P)
            adj3_sb = adjt_all[:].rearrange("p (t i) -> p t i", t=NT)
            nc.sync.dma_start(xt3[:, :, 0:NI], xbt3[:, :, 0:NI])
            nc.sync.dma_start(adj3_sb[:, 0:4, :], adj3_in[:, 0:4, :])
            nc.sync.dma_start(adj3_sb[:, 4:8, :], adj3_in[:, 4:8, :])
            nc.sync.dma_start(xt3[:, :, NI:N], xbt3[:, :, NI:N])
            nc.sync.dma_start(adj3_sb[:, 8:12, :], adj3_in[:, 8:12, :])
            nc.sync.dma_start(adj3_sb[:, 12:16, :], adj3_in[:, 12:16, :])

            # ---- s_srcT (all heads) -> E2 rows [4, NI] ----
            for c in range(2):
                pss = ps_tile([H, 512], f"pss_{c}", bank=c)
                for kt in range(KT):
                    pe(nc.tensor.matmul(
                        pss[:],
                        wta_r[kt][:, D : D + H],
                        xt_r[:, kt * N + c * 512 : kt * N + (c + 1) * 512],
                        start=(kt == 0),
                        stop=(kt == KT - 1),
                    ))
                nc.scalar.activation(
                    er4[:, c * 512 : (c + 1) * 512],
                    pss[:],
                    AF.Exp,
                    scale=-(1.0 - NEG_SLOPE),
                )

            # ---- e2rep: broadcast E2 across partitions via selector matmul;
            # all copies PSUM->SBUF on scalar (gpsimd cannot access PSUM) ----
            PSB_BANKS = {0: (0, 1), 1: (2, 3), 2: (6, 7), 3: (0, 1)}
            for h in range(H):
                for c in range(2):
                    psb = ps_tile([P, 512], f"psb_{h}_{c}", bank=PSB_BANKS[h][c])
                    pe(nc.tensor.matmul(
                        psb[:], sels[h], er4[0:H, c * 512 : (c + 1) * 512]
                    ))
                    dst = e2rep[:, h * NI + c * 512 : h * NI + (c + 1) * 512]
                    nc.scalar.activation(dst, psb[:], AF.Copy)

            # ---- h phase: psh = x @ wta ; f1/gf1/nf1 ; hplus (bf16+ones) ----
            hp4 = hplus[:].rearrange("p (t h c) -> p t h c", t=NT, h=H)
            for nt in range(NT):
                psh = ps_tile([P, WC], f"psh_{nt}", bank=4 + nt % 3)
                for kt in range(KT):
                    pe(nc.tensor.matmul(
                        psh[:],
                        xt_r[:, kt * N + nt * P : kt * N + (nt + 1) * P],
                        wta_r[kt][:],
                        start=(kt == 0),
                        stop=(kt == KT - 1),
                    ))
                sd = psh[:, D + H : D + 2 * H]
                nc.scalar.activation(
                    f1_sb[:, nt * H : (nt + 1) * H], sd, AF.Exp
                )
                nc.scalar.activation(
                    gf1_sb[:, nt * H : (nt + 1) * H], sd, AF.Exp, scale=NEG_SLOPE
                )
                nc.gpsimd.tensor_scalar(
                    nf1_sb[:, nt * H : (nt + 1) * H],
                    f1_sb[:, nt * H : (nt + 1) * H],
                    -1.0,
                    None,
                    ALU.mult,
                )
                psh4 = psh[:, 0:D].rearrange("p (h c) -> p h c", h=H)
                nc.scalar.activation(hp4[:, nt, :, 0:HD], psh4[:], AF.Copy)

            # ---- main: P^T tiles (3-engine split) + aggregation matmuls ----
            def emit_sot(h):
                for half in range(2):
                    soT = ostagep.tile(
                        [HD + 1, 512], F32, tag="soT", name=f"soT_{h}_{half}"
                    )
                    nc.scalar.activation(soT[:], psoT[h * 2 + half][:], AF.Copy)
                    nc.sync.dma_start(outs[h * 2 + half], soT[:])

            psoT = {}
            for h in range(H):
                e2h = e2rep[:, h * NI : (h + 1) * NI]
                pts = []
                for jt in range(NT):
                    path = PATHS[h][jt]
                    gf1a = gf1_sb[:, jt * H + h : jt * H + h + 1]
                    f1a = f1_sb[:, jt * H + h : jt * H + h + 1]
                    nf1a = nf1_sb[:, jt * H + h : jt * H + h + 1]
                    adjs = adjt_all[:, jt * NI : (jt + 1) * NI]
                    pt = ptp.tile([P, NI], BF16, tag="pt", name=f"pt_{h}_{jt}")
                    if path == 1:
                        t = tp.tile([P, NI], BF16, tag="t")
                        nc.scalar.activation(
                            t[:], e2h, AF.Relu, bias=nf1a, scale=gf1a
                        )
                        nc.vector.scalar_tensor_tensor(
                            pt[:], t[:], f1a, adjs, ALU.add, ALU.mult
                        )
                    else:
                        v = vp.tile([P, NI], BF16, tag="v")
                        eng = nc.vector if path == 0 else nc.gpsimd
                        eng.tensor_scalar(
                            v[:], e2h, gf1a, f1a, ALU.mult, ALU.max
                        )
                        nc.vector.tensor_tensor(pt[:], v[:], adjs, ALU.mult)
                    pts.append(pt)
                for hh in (h * 2, h * 2 + 1):
                    psoT[hh] = ps_tile([HD + 1, 512], f"psoT_{hh}", bank=hh)
                for half in range(2):
                    for jt in range(NT):
                        pe(nc.tensor.matmul(
                            psoT[h * 2 + half][:],
                            hplus[:, jt * HP + h * (HD + 1) : jt * HP + (h + 1) * (HD + 1)],
                            pts[jt][:, half * 512 : (half + 1) * 512],
                            start=(jt == 0),
                            stop=(jt == NT - 1),
                            skip_group_check=True,
                        ))
                if h >= 1:
                    emit_sot(h - 1)
            emit_sot(H - 1)

    if split_waits:
        _split_waits(nc)
    nc.finalize()
    return nc


_NC_CACHE = None


def _get_nc():
    global _NC_CACHE
    if _NC_CACHE is None:
        _NC_CACHE = build_nc()
    return _NC_CACHE


def make_in_maps(x, adj, W, a_src, a_dst):
    x = np.ascontiguousarray(x, dtype=np.float32)
    W = np.ascontiguousarray(W, dtype=np.float32)
    a_src = np.ascontiguousarray(a_src, dtype=np.float32)
    a_dst = np.ascontiguousarray(a_dst, dtype=np.float32)

    A_src = np.zeros((D, H), np.float32)
    A_dst = np.zeros((D, H), np.float32)
    for h in range(H):
        A_src[h * HD : (h + 1) * HD, h] = a_src[h]
        A_dst[h * HD : (h + 1) * HD, h] = a_dst[h]
    Wt = W.T.astype(np.float32)
    wta = np.ascontiguousarray(
        np.concatenate([Wt, Wt @ A_src, Wt @ A_dst], axis=1), dtype=np.float32
    )

    selm = np.zeros((H, H * P), ml_dtypes.bfloat16)
    for h in range(H):
        selm[h, h * P : (h + 1) * P] = 1.0

    in_maps = []
    adjT_cache = {}
    for c in range(NCORES):
        b, ihalf = c // 2, c % 2
        ilo = ihalf * NI
        if b not in adjT_cache:
            adjT_cache[b] = adj[b].astype(ml_dtypes.bfloat16).T
        # column/row permutation: the core's own i-half comes first so the
        # SPMD program can treat block 0 as "own columns" on every core.
        if ihalf == 0:
            xbt_in = x[b].T
            adjt_in = adjT_cache[b][:, ilo : ilo + NI]
        else:
            xbt_in = np.roll(x[b].T, NI, axis=1)
            adjt_in = np.roll(adjT_cache[b], NI, axis=0)[:, ilo : ilo + NI]
        in_maps.append(
            {
                "xbt": np.ascontiguousarray(xbt_in, dtype=np.float16),
                "wta": wta.astype(np.float16),
                "adjtb": np.ascontiguousarray(adjt_in),
                "selm": selm,
            }
        )
    return in_maps


def kernel(x, adj, W, a_src, a_dst):
    in_maps = make_in_maps(x, adj, W, a_src, a_dst)
    nc = _get_nc()
    res = run_bass_kernel_spmd(nc, in_maps, list(range(NCORES)))

    out = np.empty((B, N, D), np.float32)
    for c in range(NCORES):
        b, ihalf = c // 2, c % 2
        ilo = ihalf * NI
        o = np.asarray(res.results[c]["outs"], np.float32)  # [8, 65, 512]
        for h in range(H):
            for half in range(2):
                blk = o[h * 2 + half]
                quot = blk[0:HD, :] / blk[HD, :][None, :]
                out[
                    b,
                    ilo + half * 512 : ilo + (half + 1) * 512,
                    h * HD : (h + 1) * HD,
                ] = quot.T
    return out


# revision 16
# speedup vs baseline: 4.7879x; 4.7879x over previous
"""GAT layer (B=4, N=2048, D=256, H=4) on 8 trn2 NeuronCores.

Sharding: core c -> (b = c//2, i-half = c%2).  Each core computes
out[b, ihalf*1024:(ihalf+1)*1024, :]; h is computed on-device from the full
x[b] (passed pre-transposed as x[b].T, column-permuted so the core's own
i-half comes first).

Math: with z = s_src[i] + s_dst[j], the reference computes
    alpha = softmax_j(mask(leaky_relu(z)));  out = alpha @ h_head.
Softmax is shift-invariant per destination row i, so we use shifted
unnormalized weights (exact same alpha).  F1 = exp(s_dst) is folded into
the attention weights (not into h):
    P[j,i] = adj[j,i] * F1[j] * max(E2[i]*G[j], 1)
           = adj[j,i] * max(E2[i]*GF1[j], F1[j])
with GF1 = exp(0.2*s_dst), E2 = exp(-0.8*s_src)
(using exp(max(a,b)) = max(exp a, exp b) and leaky = max(z, 0.2 z)).
Row sums come from an appended ones-column in the aggregation matmul
stationary: psoT = [h_head | 1].T @ P^T; numerator and denominator are
DMA'd out raw and the final divide + transpose happens on the host.

The per-(head, j-tile) elementwise work P^T is split across three engines:
  D  tiles: DVE tensor_scalar (max(e2rep*gf1, f1)) + tensor_tensor (*adjT)
  S1 tiles: scalar ACT t=Relu(gf1*e2rep - f1); DVE STT pt=(t+f1)*adjT
  S2 tiles: scalar ACT as above; gpsimd STT
"""

import sys

for _p in ("/opt/trn_rl_repo", "/root/.axon_site/_ro/trn_rl_repo"):
    if _p not in sys.path:
        sys.path.insert(0, _p)

import ml_dtypes
import numpy as np

import concourse.bass as bass
import concourse.mybir as mybir
from concourse import tile
from concourse.bass_utils import run_bass_kernel_spmd
from concourse.vector_clock import ScopedClock

F32 = mybir.dt.float32
F16 = mybir.dt.float16
BF16 = mybir.dt.bfloat16
AF = mybir.ActivationFunctionType
ALU = mybir.AluOpType

B, N, D, H, HD = 4, 2048, 256, 4, 64
NEG_SLOPE = 0.2
P = 128
NI = N // 2          # i-rows per core (1024)
NT = N // P          # 16 j tiles
KT = D // P          # 2 k tiles
NCORES = 8
WC = D + 2 * H       # 264 aug cols: [W.T | Wt@A_src | Wt@A_dst]
HP = H * (HD + 1)    # 260 hplus cols per j-tile

# per-head split of the 16 j-tiles: first N_D are DVE tiles (TS + batched
# TT over runs of up to 4 tiles), the rest are scalar-ACT + DVE-STT singles.
N_D = [14, 6, 7, 7]


def _patch_tile_drain():
    """walrus rejects >1 sem wait on one instruction in this toolchain; split
    the TileContext tail drain's waits across consecutive SP drains."""
    if getattr(tile.TileContext, "_drain_patched", False):
        return

    def _drain_and_barrier(self, tick_clock, wait_clock):
        nc = self.nc
        drain_inst = nc.sync.drain()
        wait_clock.add_sem_waits(
            drain_inst.ins, ScopedClock({None: tick_clock.global_clock})
        )
        si = drain_inst.ins.sync_info
        waits = list(si.on_wait) if (si and si.on_wait) else []
        if len(waits) > 1:
            ups = list(si.on_update) if (si and si.on_update) else []
            drain_inst.ins.sync_info = mybir.SyncInfo(on_wait=waits[:1], on_update=ups)
            for i in range(1, len(waits)):
                extra = nc.sync.drain()
                extra.ins.sync_info = mybir.SyncInfo(
                    on_wait=waits[i : i + 1], on_update=[]
                )
        nc.all_engine_barrier()
        assert self.sems is not None
        popped = nc._tile_sem_poison_stack.pop()
        assert popped is self._sem_poison
        nc.clear_and_free_semaphores(list(self.sems.allocated().values()))
        nc.all_engine_barrier()

    tile.TileContext._drain_and_barrier = _drain_and_barrier
    tile.TileContext._drain_patched = True


def _split_waits(nc, maxw=1):
    """Hoist excess sem waits onto same-engine EventSemaphore carriers placed
    just before the instruction (same engine + program order => equivalent)."""
    n_split = 0
    for f in nc.m.functions:
        for bb in f.blocks:
            insts = list(bb.instructions)
            out = []
            changed = False
            for inst in insts:
                si = inst.sync_info
                waits = list(si.on_wait) if (si and si.on_wait) else []
                if len(waits) > maxw and inst.engine is not None:
                    changed = True
                    extra, keep = waits[:-maxw], waits[-maxw:]
                    for k in range(0, len(extra), maxw):
                        d = mybir.InstEventSemaphore(
                            name=f"{inst.name}-wsplit{k}", ins=[], outs=[]
                        )
                        d.engine = inst.engine
                        d.sync_info = mybir.SyncInfo(
                            on_wait=extra[k : k + maxw], on_update=[]
                        )
                        out.append(d)
                        n_split += 1
                    ups = list(si.on_update) if (si and si.on_update) else []
                    inst.sync_info = mybir.SyncInfo(on_wait=keep, on_update=ups)
                out.append(inst)
            if changed:
                bb.instructions = out
    return n_split


def build_nc(split_waits=True):
    _patch_tile_drain()
    nc = bass.Bass("TRN2", target_bir_lowering=False, debug=False)

    xbt = nc.dram_tensor("xbt", [D, N], F16, kind="ExternalInput")    # x[b].T perm
    wta = nc.dram_tensor("wta", [D, WC], F16, kind="ExternalInput")
    adjtb = nc.dram_tensor("adjtb", [N, NI], BF16, kind="ExternalInput")
    er4d = nc.dram_tensor("er4d", [H, NI], BF16, kind="Internal")
    outs = nc.dram_tensor("outs", [2 * H, HD + 1, 512], F32, kind="ExternalOutput")

    with tile.TileContext(nc) as tc:
        with (
            tc.tile_pool(name="const", bufs=1) as constp,
            tc.tile_pool(name="big", bufs=1) as bigp,
            tc.tile_pool(name="rows", bufs=1) as rowsp,
            tc.tile_pool(name="vqwork", bufs=3) as vqp,
            tc.tile_pool(name="twork", bufs=8) as tp,
            tc.tile_pool(name="ptq", bufs=6) as ptqp,
            tc.tile_pool(name="pts", bufs=12) as ptsp,
            tc.tile_pool(name="ostage", bufs=3) as ostagep,
            tc.tile_pool(name="psall", bufs=1, space="PSUM") as psall,
        ):
            def ps_tile(shape, name, bank):
                return psall.tile(shape, F32, tag=f"bank{bank}", name=name)

            pe_prev = [None]

            def pe(bi):
                # pin PE stream order: PSUM accumulation groups must stay
                # contiguous on PE (interleaving corrupts accumulation on HW)
                if pe_prev[0] is not None:
                    tile.add_dep_helper(bi.ins, pe_prev[0], reason="pe-order")
                pe_prev[0] = bi.ins
                return bi

            # ---- constants ----
            wta_sb = [
                constp.tile([P, WC], F16, tag=f"wta{kt}", name=f"wta_sb{kt}")
                for kt in range(KT)
            ]
            for kt in range(KT):
                nc.sync.dma_start(wta_sb[kt][:], wta[kt * P : (kt + 1) * P, :])
            wta_r = [wta_sb[kt][:] for kt in range(KT)]

            # ---- big SBUF tensors ----
            xt_raw = bigp.tile([P, KT * N], F16, tag="xtraw")
            xt_r = xt_raw[:]
            adjt_all = bigp.tile([P, NT * NI], BF16, tag="adjt")
            e2rep = bigp.tile([P, H * NI], BF16, tag="e2rep")
            hplus = bigp.tile([P, NT * HP], BF16, tag="hplus")
            f1_sb = bigp.tile([P, NT * H], F32, tag="f1")
            gf1_sb = bigp.tile([P, NT * H], F32, tag="gf1")
            nf1_sb = bigp.tile([P, NT * H], F32, tag="nf1")
            er4 = rowsp.tile([H, NI], BF16, tag="er4")

            nc.gpsimd.memset(hplus[:], 1.0)

            # ---- input DMAs (batched, own-half x first) ----
            xbt3 = xbt[:].rearrange("(k p) n -> p k n", p=P)
            xt3 = xt_raw[:].rearrange("p (k n) -> p k n", k=KT)
            adj3_in = adjtb[:].rearrange("(t p) i -> p t i", p=P)
            adj3_sb = adjt_all[:].rearrange("p (t i) -> p t i", t=NT)
            nc.sync.dma_start(xt3[:, :, 0:NI], xbt3[:, :, 0:NI])
            nc.sync.dma_start(adj3_sb[:, 0:4, :], adj3_in[:, 0:4, :])
            nc.sync.dma_start(adj3_sb[:, 4:8, :], adj3_in[:, 4:8, :])
            nc.sync.dma_start(xt3[:, :, NI:N], xbt3[:, :, NI:N])
            nc.sync.dma_start(adj3_sb[:, 8:12, :], adj3_in[:, 8:12, :])
            nc.sync.dma_start(adj3_sb[:, 12:16, :], adj3_in[:, 12:16, :])

            # ---- s_srcT (all heads) -> E2 rows [4, NI] ----
            for c in range(2):
                pss = ps_tile([H, 512], f"pss_{c}", bank=c)
                for kt in range(KT):
                    pe(nc.tensor.matmul(
                        pss[:],
                        wta_r[kt][:, D : D + H],
                        xt_r[:, kt * N + c * 512 : kt * N + (c + 1) * 512],
                        start=(kt == 0),
                        stop=(kt == KT - 1),
                    ))
                nc.scalar.activation(
                    er4[:, c * 512 : (c + 1) * 512],
                    pss[:],
                    AF.Exp,
                    scale=-(1.0 - NEG_SLOPE),
                )

            # ---- e2rep: broadcast E2 across partitions via a DRAM round
            # trip (DMA engines replicate a DRAM row; compute engines cannot
            # partition-broadcast and gpsimd is too slow) ----
            nc.sync.dma_start(er4d[:], er4[:])
            for h in range(H):
                nc.sync.dma_start(
                    e2rep[:, h * NI : (h + 1) * NI],
                    er4d[h : h + 1, :].to_broadcast((P, NI)),
                )

            # ---- h phase: psh = x @ wta ; f1/gf1/nf1 ; hplus (bf16+ones) ----
            hp4 = hplus[:].rearrange("p (t h c) -> p t h c", t=NT, h=H)
            for nt in range(NT):
                psh = ps_tile([P, WC], f"psh_{nt}", bank=4 + nt % 3)
                for kt in range(KT):
                    pe(nc.tensor.matmul(
                        psh[:],
                        xt_r[:, kt * N + nt * P : kt * N + (nt + 1) * P],
                        wta_r[kt][:],
                        start=(kt == 0),
                        stop=(kt == KT - 1),
                    ))
                sd = psh[:, D + H : D + 2 * H]
                nc.scalar.activation(
                    f1_sb[:, nt * H : (nt + 1) * H], sd, AF.Exp
                )
                nc.scalar.activation(
                    gf1_sb[:, nt * H : (nt + 1) * H], sd, AF.Exp, scale=NEG_SLOPE
                )
                nc.gpsimd.tensor_scalar(
                    nf1_sb[:, nt * H : (nt + 1) * H],
                    f1_sb[:, nt * H : (nt + 1) * H],
                    -1.0,
                    None,
                    ALU.mult,
                )
                psh4 = psh[:, 0:D].rearrange("p (h c) -> p h c", h=H)
                nc.scalar.activation(hp4[:, nt, :, 0:HD], psh4[:], AF.Copy)

            # ---- main: P^T tiles (3-engine split) + aggregation matmuls ----
            def emit_sot(h):
                for half in range(2):
                    soT = ostagep.tile(
                        [HD + 1, 512], F32, tag="soT", name=f"soT_{h}_{half}"
                    )
                    nc.scalar.activation(soT[:], psoT[h * 2 + half][:], AF.Copy)
                    nc.sync.dma_start(outs[h * 2 + half], soT[:])

            psoT = {}
            for h in range(H):
                e2h = e2rep[:, h * NI : (h + 1) * NI]
                nd = N_D[h]
                pt_map = {}
                for base in range(0, nd, 4):
                    L = min(4, nd - base)
                    vq = vqp.tile([P, L * NI], BF16, tag="vq")
                    for i in range(L):
                        jt = base + i
                        nc.vector.tensor_scalar(
                            vq[:, i * NI : (i + 1) * NI],
                            e2h,
                            gf1_sb[:, jt * H + h : jt * H + h + 1],
                            f1_sb[:, jt * H + h : jt * H + h + 1],
                            ALU.mult,
                            ALU.max,
                        )
                    ptq = ptqp.tile([P, L * NI], BF16, tag="ptq", name=f"ptq_{h}_{base}")
                    nc.vector.tensor_tensor(
                        ptq[:], vq[:], adjt_all[:, base * NI : (base + L) * NI], ALU.mult
                    )
                    for i in range(L):
                        pt_map[base + i] = (ptq, i)
                for jt in range(nd, NT):
                    t = tp.tile([P, NI], BF16, tag="t")
                    nc.scalar.activation(
                        t[:],
                        e2h,
                        AF.Relu,
                        bias=nf1_sb[:, jt * H + h : jt * H + h + 1],
                        scale=gf1_sb[:, jt * H + h : jt * H + h + 1],
                    )
                    pt = ptsp.tile([P, NI], BF16, tag="pt", name=f"pt_{h}_{jt}")
                    nc.vector.scalar_tensor_tensor(
                        pt[:],
                        t[:],
                        f1_sb[:, jt * H + h : jt * H + h + 1],
                        adjt_all[:, jt * NI : (jt + 1) * NI],
                        ALU.add,
                        ALU.mult,
                    )
                    pt_map[jt] = (pt, 0)
                for hh in (h * 2, h * 2 + 1):
                    psoT[hh] = ps_tile([HD + 1, 512], f"psoT_{hh}", bank=hh)
                for half in range(2):
                    for jt in range(NT):
                        buf, i = pt_map[jt]
                        pe(nc.tensor.matmul(
                            psoT[h * 2 + half][:],
                            hplus[:, jt * HP + h * (HD + 1) : jt * HP + (h + 1) * (HD + 1)],
                            buf[:, i * NI + half * 512 : i * NI + half * 512 + 512],
                            start=(jt == 0),
                            stop=(jt == NT - 1),
                            skip_group_check=True,
                        ))
                if h >= 1:
                    emit_sot(h - 1)
            emit_sot(H - 1)

    if split_waits:
        _split_waits(nc)
    nc.finalize()
    return nc


_NC_CACHE = None


def _get_nc():
    global _NC_CACHE
    if _NC_CACHE is None:
        _NC_CACHE = build_nc()
    return _NC_CACHE


def make_in_maps(x, adj, W, a_src, a_dst):
    x = np.ascontiguousarray(x, dtype=np.float32)
    W = np.ascontiguousarray(W, dtype=np.float32)
    a_src = np.ascontiguousarray(a_src, dtype=np.float32)
    a_dst = np.ascontiguousarray(a_dst, dtype=np.float32)

    A_src = np.zeros((D, H), np.float32)
    A_dst = np.zeros((D, H), np.float32)
    for h in range(H):
        A_src[h * HD : (h + 1) * HD, h] = a_src[h]
        A_dst[h * HD : (h + 1) * HD, h] = a_dst[h]
    Wt = W.T.astype(np.float32)
    wta = np.ascontiguousarray(
        np.concatenate([Wt, Wt @ A_src, Wt @ A_dst], axis=1), dtype=np.float32
    )

    in_maps = []
    adjT_cache = {}
    for c in range(NCORES):
        b, ihalf = c // 2, c % 2
        ilo = ihalf * NI
        if b not in adjT_cache:
            adjT_cache[b] = adj[b].astype(ml_dtypes.bfloat16).T
        # column/row permutation: the core's own i-half comes first so the
        # SPMD program can treat block 0 as "own columns" on every core.
        if ihalf == 0:
            xbt_in = x[b].T
            adjt_in = adjT_cache[b][:, ilo : ilo + NI]
        else:
            xbt_in = np.roll(x[b].T, NI, axis=1)
            adjt_in = np.roll(adjT_cache[b], NI, axis=0)[:, ilo : ilo + NI]
        in_maps.append(
            {
                "xbt": np.ascontiguousarray(xbt_in, dtype=np.float16),
                "wta": wta.astype(np.float16),
                "adjtb": np.ascontiguousarray(adjt_in),
            }
        )
    return in_maps


def kernel(x, adj, W, a_src, a_dst):
    in_maps = make_in_maps(x, adj, W, a_src, a_dst)
    nc = _get_nc()
    res = run_bass_kernel_spmd(nc, in_maps, list(range(NCORES)))

    out = np.empty((B, N, D), np.float32)
    for c in range(NCORES):
        b, ihalf = c // 2, c % 2
        ilo = ihalf * NI
        o = np.asarray(res.results[c]["outs"], np.float32)  # [8, 65, 512]
        for h in range(H):
            for half in range(2):
                blk = o[h * 2 + half]
                quot = blk[0:HD, :] / blk[HD, :][None, :]
                out[
                    b,
                    ilo + half * 512 : ilo + (half + 1) * 512,
                    h * HD : (h + 1) * HD,
                ] = quot.T
    return out


# revision 17
# speedup vs baseline: 5.4230x; 1.1326x over previous
"""GAT layer (B=4, N=2048, D=256, H=4) on 8 trn2 NeuronCores.

Sharding: core c -> (b = c//2, i-half = c%2).  Each core computes
out[b, ihalf*1024:(ihalf+1)*1024, :]; h is computed on-device from the full
x[b] (passed pre-transposed as x[b].T, column-permuted so the core's own
i-half comes first).

Math: with z = s_src[i] + s_dst[j], the reference computes
    alpha = softmax_j(mask(leaky_relu(z)));  out = alpha @ h_head.
Softmax is shift-invariant per destination row i, so we use shifted
unnormalized weights (exact same alpha).  F1 = exp(s_dst) is folded into
the attention weights (not into h):
    P[j,i] = adj[j,i] * F1[j] * max(E2[i]*G[j], 1)
           = adj[j,i] * max(E2[i]*GF1[j], F1[j])
with GF1 = exp(0.2*s_dst), E2 = exp(-0.8*s_src)
(using exp(max(a,b)) = max(exp a, exp b) and leaky = max(z, 0.2 z)).
Row sums come from an appended ones-column in the aggregation matmul
stationary: psoT = [h_head | 1].T @ P^T; numerator and denominator are
DMA'd out raw and the final divide + transpose happens on the host.

The per-(head, j-tile) elementwise work P^T is split across three engines:
  D  tiles: DVE tensor_scalar (max(e2rep*gf1, f1)) + tensor_tensor (*adjT)
  S1 tiles: scalar ACT t=Relu(gf1*e2rep - f1); DVE STT pt=(t+f1)*adjT
  S2 tiles: scalar ACT as above; gpsimd STT
"""

import sys

for _p in ("/opt/trn_rl_repo", "/root/.axon_site/_ro/trn_rl_repo"):
    if _p not in sys.path:
        sys.path.insert(0, _p)

import ml_dtypes
import numpy as np

import concourse.bass as bass
import concourse.mybir as mybir
from concourse import tile
from concourse.bass_utils import run_bass_kernel_spmd
from concourse.vector_clock import ScopedClock

F32 = mybir.dt.float32
F16 = mybir.dt.float16
BF16 = mybir.dt.bfloat16
AF = mybir.ActivationFunctionType
ALU = mybir.AluOpType

B, N, D, H, HD = 4, 2048, 256, 4, 64
NEG_SLOPE = 0.2
P = 128
NI = N // 2          # i-rows per core (1024)
NT = N // P          # 16 j tiles
KT = D // P          # 2 k tiles
NCORES = 8
WC = D + 2 * H       # 264 aug cols: [W.T | Wt@A_src | Wt@A_dst]
HP = H * (HD + 1)    # 260 hplus cols per j-tile

# All elementwise tiles run on the DVE: per-tile tensor_scalar (2x mode)
# plus one batched tensor_tensor mask-multiply per run of 4 j-tiles.


def _patch_tile_drain():
    """walrus rejects >1 sem wait on one instruction in this toolchain; split
    the TileContext tail drain's waits across consecutive SP drains."""
    if getattr(tile.TileContext, "_drain_patched", False):
        return

    def _drain_and_barrier(self, tick_clock, wait_clock):
        nc = self.nc
        drain_inst = nc.sync.drain()
        wait_clock.add_sem_waits(
            drain_inst.ins, ScopedClock({None: tick_clock.global_clock})
        )
        si = drain_inst.ins.sync_info
        waits = list(si.on_wait) if (si and si.on_wait) else []
        if len(waits) > 1:
            ups = list(si.on_update) if (si and si.on_update) else []
            drain_inst.ins.sync_info = mybir.SyncInfo(on_wait=waits[:1], on_update=ups)
            for i in range(1, len(waits)):
                extra = nc.sync.drain()
                extra.ins.sync_info = mybir.SyncInfo(
                    on_wait=waits[i : i + 1], on_update=[]
                )
        nc.all_engine_barrier()
        assert self.sems is not None
        popped = nc._tile_sem_poison_stack.pop()
        assert popped is self._sem_poison
        nc.clear_and_free_semaphores(list(self.sems.allocated().values()))
        nc.all_engine_barrier()

    tile.TileContext._drain_and_barrier = _drain_and_barrier
    tile.TileContext._drain_patched = True


def _split_waits(nc, maxw=1):
    """Hoist excess sem waits onto same-engine EventSemaphore carriers placed
    just before the instruction (same engine + program order => equivalent)."""
    n_split = 0
    for f in nc.m.functions:
        for bb in f.blocks:
            insts = list(bb.instructions)
            out = []
            changed = False
            for inst in insts:
                si = inst.sync_info
                waits = list(si.on_wait) if (si and si.on_wait) else []
                if len(waits) > maxw and inst.engine is not None:
                    changed = True
                    extra, keep = waits[:-maxw], waits[-maxw:]
                    for k in range(0, len(extra), maxw):
                        d = mybir.InstEventSemaphore(
                            name=f"{inst.name}-wsplit{k}", ins=[], outs=[]
                        )
                        d.engine = inst.engine
                        d.sync_info = mybir.SyncInfo(
                            on_wait=extra[k : k + maxw], on_update=[]
                        )
                        out.append(d)
                        n_split += 1
                    ups = list(si.on_update) if (si and si.on_update) else []
                    inst.sync_info = mybir.SyncInfo(on_wait=keep, on_update=ups)
                out.append(inst)
            if changed:
                bb.instructions = out
    return n_split


def build_nc(split_waits=True):
    _patch_tile_drain()
    nc = bass.Bass("TRN2", target_bir_lowering=False, debug=False)

    xbt = nc.dram_tensor("xbt", [D, N], F16, kind="ExternalInput")    # x[b].T perm
    wta = nc.dram_tensor("wta", [D, WC], F16, kind="ExternalInput")
    adjtb = nc.dram_tensor("adjtb", [N, NI], BF16, kind="ExternalInput")
    selm = nc.dram_tensor("selm", [H, H * P], BF16, kind="ExternalInput")
    outs = nc.dram_tensor("outs", [2 * H, HD + 1, 512], BF16, kind="ExternalOutput")

    with tile.TileContext(nc) as tc:
        with (
            tc.tile_pool(name="const", bufs=1) as constp,
            tc.tile_pool(name="big", bufs=1) as bigp,
            tc.tile_pool(name="rows", bufs=1) as rowsp,
            tc.tile_pool(name="vqwork", bufs=3) as vqp,
            tc.tile_pool(name="ptq", bufs=6) as ptqp,
            tc.tile_pool(name="ostage", bufs=3) as ostagep,
            tc.tile_pool(name="psall", bufs=1, space="PSUM") as psall,
        ):
            def ps_tile(shape, name, bank):
                return psall.tile(shape, F32, tag=f"bank{bank}", name=name)

            pe_prev = [None]

            def pe(bi):
                # pin PE stream order: PSUM accumulation groups must stay
                # contiguous on PE (interleaving corrupts accumulation on HW)
                if pe_prev[0] is not None:
                    tile.add_dep_helper(bi.ins, pe_prev[0], reason="pe-order")
                pe_prev[0] = bi.ins
                return bi

            # ---- constants ----
            wta_sb = [
                constp.tile([P, WC], F16, tag=f"wta{kt}", name=f"wta_sb{kt}")
                for kt in range(KT)
            ]
            sel_sb = constp.tile([H, H * P], BF16, tag="selm")
            nc.sync.dma_start(sel_sb[:], selm[:])
            sels = [sel_sb[:, h * P : (h + 1) * P] for h in range(H)]
            for kt in range(KT):
                nc.sync.dma_start(wta_sb[kt][:], wta[kt * P : (kt + 1) * P, :])
            wta_r = [wta_sb[kt][:] for kt in range(KT)]

            # ---- big SBUF tensors ----
            xt_raw = bigp.tile([P, KT * N], F16, tag="xtraw")
            xt_r = xt_raw[:]
            adjt_all = bigp.tile([P, NT * NI], BF16, tag="adjt")
            e2rep = bigp.tile([P, H * NI], BF16, tag="e2rep")
            hplus = bigp.tile([P, NT * HP], BF16, tag="hplus")
            f1_sb = bigp.tile([P, NT * H], F32, tag="f1")
            gf1_sb = bigp.tile([P, NT * H], F32, tag="gf1")
            er4 = rowsp.tile([H, NI], BF16, tag="er4")

            nc.gpsimd.memset(hplus[:], 1.0)

            # ---- input DMAs (batched, own-half x first) ----
            xbt3 = xbt[:].rearrange("(k p) n -> p k n", p=P)
            xt3 = xt_raw[:].rearrange("p (k n) -> p k n", k=KT)
            adj3_in = adjtb[:].rearrange("(t p) i -> p t i", p=P)
            adj3_sb = adjt_all[:].rearrange("p (t i) -> p t i", t=NT)
            nc.sync.dma_start(xt3[:, :, 0:NI], xbt3[:, :, 0:NI])
            nc.sync.dma_start(adj3_sb[:, 0:4, :], adj3_in[:, 0:4, :])
            nc.sync.dma_start(adj3_sb[:, 4:8, :], adj3_in[:, 4:8, :])
            nc.sync.dma_start(xt3[:, :, NI:N], xbt3[:, :, NI:N])
            nc.sync.dma_start(adj3_sb[:, 8:12, :], adj3_in[:, 8:12, :])
            nc.sync.dma_start(adj3_sb[:, 12:16, :], adj3_in[:, 12:16, :])

            # ---- s_srcT (all heads) -> E2 rows [4, NI] ----
            for c in range(2):
                pss = ps_tile([H, 512], f"pss_{c}", bank=c)
                for kt in range(KT):
                    pe(nc.tensor.matmul(
                        pss[:],
                        wta_r[kt][:, D : D + H],
                        xt_r[:, kt * N + c * 512 : kt * N + (c + 1) * 512],
                        start=(kt == 0),
                        stop=(kt == KT - 1),
                    ))
                nc.scalar.activation(
                    er4[:, c * 512 : (c + 1) * 512],
                    pss[:],
                    AF.Exp,
                    scale=-(1.0 - NEG_SLOPE),
                )

            # ---- e2rep: broadcast E2 across partitions via a selector
            # matmul on PSUM bank 7 + scalar copy; h1..h3 are emitted from
            # inside the nt loop to interleave with the psh stream ----
            def emit_e2rep(h):
                for c in range(2):
                    psb = ps_tile([P, 512], f"psb_{h}_{c}", bank=7)
                    pe(nc.tensor.matmul(
                        psb[:], sels[h], er4[0:H, c * 512 : (c + 1) * 512]
                    ))
                    nc.scalar.activation(
                        e2rep[:, h * NI + c * 512 : h * NI + (c + 1) * 512],
                        psb[:],
                        AF.Copy,
                    )

            emit_e2rep(0)

            # ---- h phase: psh = x @ wta ; f1/gf1/nf1 ; hplus (bf16+ones) ----
            hp4 = hplus[:].rearrange("p (t h c) -> p t h c", t=NT, h=H)
            for nt in range(NT):
                psh = ps_tile([P, WC], f"psh_{nt}", bank=4 + nt % 3)
                for kt in range(KT):
                    pe(nc.tensor.matmul(
                        psh[:],
                        xt_r[:, kt * N + nt * P : kt * N + (nt + 1) * P],
                        wta_r[kt][:],
                        start=(kt == 0),
                        stop=(kt == KT - 1),
                    ))
                sd = psh[:, D + H : D + 2 * H]
                nc.scalar.activation(
                    f1_sb[:, nt * H : (nt + 1) * H], sd, AF.Exp
                )
                nc.scalar.activation(
                    gf1_sb[:, nt * H : (nt + 1) * H], sd, AF.Exp, scale=NEG_SLOPE
                )
                if nt in (3, 7, 11):
                    emit_e2rep(nt // 4 + 1)
                psh4 = psh[:, 0:D].rearrange("p (h c) -> p h c", h=H)
                nc.scalar.activation(hp4[:, nt, :, 0:HD], psh4[:], AF.Copy)

            # ---- main: P^T tiles (3-engine split) + aggregation matmuls ----
            def emit_sot(h, half):
                soT = ostagep.tile(
                    [HD + 1, 512], BF16, tag="soT", name=f"soT_{h}_{half}"
                )
                nc.scalar.activation(soT[:], psoT[h * 2 + half][:], AF.Copy)
                nc.sync.dma_start(outs[h * 2 + half], soT[:])

            psoT = {}
            for h in range(H):
                e2h = e2rep[:, h * NI : (h + 1) * NI]
                ptqs = []
                for base in range(0, NT, 4):
                    vq = vqp.tile([P, 4 * NI], BF16, tag="vq")
                    for i in range(4):
                        jt = base + i
                        nc.vector.tensor_scalar(
                            vq[:, i * NI : (i + 1) * NI],
                            e2h,
                            gf1_sb[:, jt * H + h : jt * H + h + 1],
                            f1_sb[:, jt * H + h : jt * H + h + 1],
                            ALU.mult,
                            ALU.max,
                        )
                    ptq = ptqp.tile([P, 4 * NI], BF16, tag="ptq", name=f"ptq_{h}_{base}")
                    nc.vector.tensor_tensor(
                        ptq[:], vq[:], adjt_all[:, base * NI : (base + 4) * NI], ALU.mult
                    )
                    ptqs.append(ptq)
                for hh in (h * 2, h * 2 + 1):
                    psoT[hh] = ps_tile([HD + 1, 512], f"psoT_{hh}", bank=hh)
                for half in range(2):
                    for jt in range(NT):
                        buf = ptqs[jt // 4]
                        i = jt % 4
                        pe(nc.tensor.matmul(
                            psoT[h * 2 + half][:],
                            hplus[:, jt * HP + h * (HD + 1) : jt * HP + (h + 1) * (HD + 1)],
                            buf[:, i * NI + half * 512 : i * NI + half * 512 + 512],
                            start=(jt == 0),
                            stop=(jt == NT - 1),
                            skip_group_check=True,
                        ))
                    emit_sot(h, half)

    if split_waits:
        _split_waits(nc)
    nc.finalize()
    return nc


_NC_CACHE = None


def _get_nc():
    global _NC_CACHE
    if _NC_CACHE is None:
        _NC_CACHE = build_nc()
    return _NC_CACHE


def make_in_maps(x, adj, W, a_src, a_dst):
    x = np.ascontiguousarray(x, dtype=np.float32)
    W = np.ascontiguousarray(W, dtype=np.float32)
    a_src = np.ascontiguousarray(a_src, dtype=np.float32)
    a_dst = np.ascontiguousarray(a_dst, dtype=np.float32)

    A_src = np.zeros((D, H), np.float32)
    A_dst = np.zeros((D, H), np.float32)
    for h in range(H):
        A_src[h * HD : (h + 1) * HD, h] = a_src[h]
        A_dst[h * HD : (h + 1) * HD, h] = a_dst[h]
    Wt = W.T.astype(np.float32)
    wta = np.ascontiguousarray(
        np.concatenate([Wt, Wt @ A_src, Wt @ A_dst], axis=1), dtype=np.float32
    )

    selm = np.zeros((H, H * P), ml_dtypes.bfloat16)
    for h in range(H):
        selm[h, h * P : (h + 1) * P] = 1.0

    in_maps = []
    adjT_cache = {}
    for c in range(NCORES):
        b, ihalf = c // 2, c % 2
        ilo = ihalf * NI
        if b not in adjT_cache:
            adjT_cache[b] = adj[b].astype(ml_dtypes.bfloat16).T
        # column/row permutation: the core's own i-half comes first so the
        # SPMD program can treat block 0 as "own columns" on every core.
        if ihalf == 0:
            xbt_in = x[b].T
            adjt_in = adjT_cache[b][:, ilo : ilo + NI]
        else:
            xbt_in = np.roll(x[b].T, NI, axis=1)
            adjt_in = np.roll(adjT_cache[b], NI, axis=0)[:, ilo : ilo + NI]
        in_maps.append(
            {
                "xbt": np.ascontiguousarray(xbt_in, dtype=np.float16),
                "wta": wta.astype(np.float16),
                "adjtb": np.ascontiguousarray(adjt_in),
                "selm": selm,
            }
        )
    return in_maps


def kernel(x, adj, W, a_src, a_dst):
    in_maps = make_in_maps(x, adj, W, a_src, a_dst)
    nc = _get_nc()
    res = run_bass_kernel_spmd(nc, in_maps, list(range(NCORES)))

    out = np.empty((B, N, D), np.float32)
    for c in range(NCORES):
        b, ihalf = c // 2, c % 2
        ilo = ihalf * NI
        o = np.asarray(res.results[c]["outs"], np.float32)  # [8, 65, 512]
        for h in range(H):
            for half in range(2):
                blk = o[h * 2 + half]
                quot = blk[0:HD, :] / blk[HD, :][None, :]
                out[
                    b,
                    ilo + half * 512 : ilo + (half + 1) * 512,
                    h * HD : (h + 1) * HD,
                ] = quot.T
    return out


# revision 19
# speedup vs baseline: 5.4607x; 1.0070x over previous
"""GAT layer (B=4, N=2048, D=256, H=4) on 8 trn2 NeuronCores.

Sharding: core c -> (b = c//2, i-half = c%2).  Each core computes
out[b, ihalf*1024:(ihalf+1)*1024, :]; h is computed on-device from the full
x[b] (passed pre-transposed as x[b].T, column-permuted so the core's own
i-half comes first).

Math: with z = s_src[i] + s_dst[j], the reference computes
    alpha = softmax_j(mask(leaky_relu(z)));  out = alpha @ h_head.
Softmax is shift-invariant per destination row i, so we use shifted
unnormalized weights (exact same alpha).  F1 = exp(s_dst) is folded into
the attention weights (not into h):
    P[j,i] = adj[j,i] * F1[j] * max(E2[i]*G[j], 1)
           = adj[j,i] * max(E2[i]*GF1[j], F1[j])
with GF1 = exp(0.2*s_dst), E2 = exp(-0.8*s_src)
(using exp(max(a,b)) = max(exp a, exp b) and leaky = max(z, 0.2 z)).
Row sums come from an appended ones-column in the aggregation matmul
stationary: psoT = [h_head | 1].T @ P^T; numerator and denominator are
DMA'd out raw and the final divide + transpose happens on the host.

The per-(head, j-tile) elementwise work P^T is split across three engines:
  D  tiles: DVE tensor_scalar (max(e2rep*gf1, f1)) + tensor_tensor (*adjT)
  S1 tiles: scalar ACT t=Relu(gf1*e2rep - f1); DVE STT pt=(t+f1)*adjT
  S2 tiles: scalar ACT as above; gpsimd STT
"""

import sys

for _p in ("/opt/trn_rl_repo", "/root/.axon_site/_ro/trn_rl_repo"):
    if _p not in sys.path:
        sys.path.insert(0, _p)

import ml_dtypes
import numpy as np

import concourse.bass as bass
import concourse.mybir as mybir
from concourse import tile
from concourse.bass_utils import run_bass_kernel_spmd
from concourse.vector_clock import ScopedClock

F32 = mybir.dt.float32
F16 = mybir.dt.float16
BF16 = mybir.dt.bfloat16
AF = mybir.ActivationFunctionType
ALU = mybir.AluOpType

B, N, D, H, HD = 4, 2048, 256, 4, 64
NEG_SLOPE = 0.2
P = 128
NI = N // 2          # i-rows per core (1024)
NT = N // P          # 16 j tiles
KT = D // P          # 2 k tiles
NCORES = 8
WC = D + 2 * H       # 264 aug cols: [W.T | Wt@A_src | Wt@A_dst]
HP = H * (HD + 1)    # 260 hplus cols per j-tile

# All elementwise tiles run on the DVE: per-tile tensor_scalar (2x mode)
# plus one batched tensor_tensor mask-multiply per run of 4 j-tiles.


def _patch_tile_drain():
    """walrus rejects >1 sem wait on one instruction in this toolchain; split
    the TileContext tail drain's waits across consecutive SP drains."""
    if getattr(tile.TileContext, "_drain_patched", False):
        return

    def _drain_and_barrier(self, tick_clock, wait_clock):
        nc = self.nc
        drain_inst = nc.sync.drain()
        wait_clock.add_sem_waits(
            drain_inst.ins, ScopedClock({None: tick_clock.global_clock})
        )
        si = drain_inst.ins.sync_info
        waits = list(si.on_wait) if (si and si.on_wait) else []
        if len(waits) > 1:
            ups = list(si.on_update) if (si and si.on_update) else []
            drain_inst.ins.sync_info = mybir.SyncInfo(on_wait=waits[:1], on_update=ups)
            for i in range(1, len(waits)):
                extra = nc.sync.drain()
                extra.ins.sync_info = mybir.SyncInfo(
                    on_wait=waits[i : i + 1], on_update=[]
                )
        nc.all_engine_barrier()
        assert self.sems is not None
        popped = nc._tile_sem_poison_stack.pop()
        assert popped is self._sem_poison
        nc.clear_and_free_semaphores(list(self.sems.allocated().values()))
        nc.all_engine_barrier()

    tile.TileContext._drain_and_barrier = _drain_and_barrier
    tile.TileContext._drain_patched = True


def _split_waits(nc, maxw=1):
    """Hoist excess sem waits onto same-engine EventSemaphore carriers placed
    just before the instruction (same engine + program order => equivalent)."""
    n_split = 0
    for f in nc.m.functions:
        for bb in f.blocks:
            insts = list(bb.instructions)
            out = []
            changed = False
            for inst in insts:
                si = inst.sync_info
                waits = list(si.on_wait) if (si and si.on_wait) else []
                if len(waits) > maxw and inst.engine is not None:
                    changed = True
                    extra, keep = waits[:-maxw], waits[-maxw:]
                    for k in range(0, len(extra), maxw):
                        d = mybir.InstEventSemaphore(
                            name=f"{inst.name}-wsplit{k}", ins=[], outs=[]
                        )
                        d.engine = inst.engine
                        d.sync_info = mybir.SyncInfo(
                            on_wait=extra[k : k + maxw], on_update=[]
                        )
                        out.append(d)
                        n_split += 1
                    ups = list(si.on_update) if (si and si.on_update) else []
                    inst.sync_info = mybir.SyncInfo(on_wait=keep, on_update=ups)
                out.append(inst)
            if changed:
                bb.instructions = out
    return n_split


def build_nc(split_waits=True):
    _patch_tile_drain()
    nc = bass.Bass("TRN2", target_bir_lowering=False, debug=False)

    xbt = nc.dram_tensor("xbt", [D, N], F16, kind="ExternalInput")    # x[b].T perm
    wta = nc.dram_tensor("wta", [D, WC], F16, kind="ExternalInput")
    adjtb = nc.dram_tensor("adjtb", [N, NI], BF16, kind="ExternalInput")
    selm = nc.dram_tensor("selm", [H, H * P], BF16, kind="ExternalInput")
    outs = nc.dram_tensor("outs", [2 * H, HD + 1, 512], BF16, kind="ExternalOutput")

    with tile.TileContext(nc) as tc:
        with (
            tc.tile_pool(name="const", bufs=1) as constp,
            tc.tile_pool(name="big", bufs=1) as bigp,
            tc.tile_pool(name="rows", bufs=1) as rowsp,
            tc.tile_pool(name="vqwork", bufs=3) as vqp,
            tc.tile_pool(name="ptq", bufs=6) as ptqp,
            tc.tile_pool(name="ostage", bufs=3) as ostagep,
            tc.tile_pool(name="psall", bufs=1, space="PSUM") as psall,
        ):
            def ps_tile(shape, name, bank):
                return psall.tile(shape, F32, tag=f"bank{bank}", name=name)

            pe_prev = [None]

            def pe(bi):
                # pin PE stream order: PSUM accumulation groups must stay
                # contiguous on PE (interleaving corrupts accumulation on HW)
                if pe_prev[0] is not None:
                    tile.add_dep_helper(bi.ins, pe_prev[0], reason="pe-order")
                pe_prev[0] = bi.ins
                return bi

            # ---- constants ----
            wta_sb = [
                constp.tile([P, WC], F16, tag=f"wta{kt}", name=f"wta_sb{kt}")
                for kt in range(KT)
            ]
            for kt in range(KT):
                nc.sync.dma_start(wta_sb[kt][:], wta[kt * P : (kt + 1) * P, :])
            sel_sb = constp.tile([H, H * P], BF16, tag="selm")
            nc.sync.dma_start(sel_sb[:], selm[:])
            sels = [sel_sb[:, h * P : (h + 1) * P] for h in range(H)]
            wta_r = [wta_sb[kt][:] for kt in range(KT)]

            # ---- big SBUF tensors ----
            xt_raw = bigp.tile([P, KT * N], F16, tag="xtraw")
            xt_r = xt_raw[:]
            adjt_all = bigp.tile([P, NT * NI], BF16, tag="adjt")
            e2rep = bigp.tile([P, H * NI], BF16, tag="e2rep")
            hplus = bigp.tile([P, NT * HP], BF16, tag="hplus")
            f1_sb = bigp.tile([P, NT * H], F32, tag="f1")
            gf1_sb = bigp.tile([P, NT * H], F32, tag="gf1")
            er4 = rowsp.tile([H, NI], BF16, tag="er4")

            nc.gpsimd.memset(hplus[:], 1.0)

            # ---- input DMAs (batched; own-half x first, split so the
            # s_src -> er4 -> e2rep chain unblocks as early as possible) ----
            xbt3 = xbt[:].rearrange("(k p) n -> p k n", p=P)
            xt3 = xt_raw[:].rearrange("p (k n) -> p k n", k=KT)
            adj3_in = adjtb[:].rearrange("(t p) i -> p t i", p=P)
            adj3_sb = adjt_all[:].rearrange("p (t i) -> p t i", t=NT)
            nc.sync.dma_start(xt3[:, :, 0:512], xbt3[:, :, 0:512])
            nc.sync.dma_start(xt3[:, :, 512:NI], xbt3[:, :, 512:NI])
            nc.sync.dma_start(adj3_sb[:, 0:4, :], adj3_in[:, 0:4, :])
            nc.sync.dma_start(adj3_sb[:, 4:8, :], adj3_in[:, 4:8, :])
            nc.sync.dma_start(xt3[:, :, NI:N], xbt3[:, :, NI:N])
            nc.sync.dma_start(adj3_sb[:, 8:12, :], adj3_in[:, 8:12, :])
            nc.sync.dma_start(adj3_sb[:, 12:16, :], adj3_in[:, 12:16, :])

            # ---- s_srcT (all heads) -> E2 rows [4, NI] ----
            for c in range(2):
                pss = ps_tile([H, 512], f"pss_{c}", bank=c)
                for kt in range(KT):
                    pe(nc.tensor.matmul(
                        pss[:],
                        wta_r[kt][:, D : D + H],
                        xt_r[:, kt * N + c * 512 : kt * N + (c + 1) * 512],
                        start=(kt == 0),
                        stop=(kt == KT - 1),
                    ))
                nc.scalar.activation(
                    er4[:, c * 512 : (c + 1) * 512],
                    pss[:],
                    AF.Exp,
                    scale=-(1.0 - NEG_SLOPE),
                )

            # ---- e2rep: broadcast E2 across partitions via a selector
            # matmul on PSUM bank 7 + scalar copy; h1..h3 are emitted from
            # inside the nt loop to interleave with the psh stream ----
            def emit_e2rep(h):
                for c in range(2):
                    psb = ps_tile([P, 512], f"psb_{h}_{c}", bank=7)
                    pe(nc.tensor.matmul(
                        psb[:], sels[h], er4[0:H, c * 512 : (c + 1) * 512]
                    ))
                    nc.scalar.activation(
                        e2rep[:, h * NI + c * 512 : h * NI + (c + 1) * 512],
                        psb[:],
                        AF.Copy,
                    )

            emit_e2rep(0)

            # ---- h phase: psh = x @ wta ; f1/gf1/nf1 ; hplus (bf16+ones) ----
            hp4 = hplus[:].rearrange("p (t h c) -> p t h c", t=NT, h=H)
            for nt in range(NT):
                psh = ps_tile([P, WC], f"psh_{nt}", bank=4 + nt % 3)
                for kt in range(KT):
                    pe(nc.tensor.matmul(
                        psh[:],
                        xt_r[:, kt * N + nt * P : kt * N + (nt + 1) * P],
                        wta_r[kt][:],
                        start=(kt == 0),
                        stop=(kt == KT - 1),
                    ))
                sd = psh[:, D + H : D + 2 * H]
                nc.scalar.activation(
                    f1_sb[:, nt * H : (nt + 1) * H], sd, AF.Exp
                )
                nc.scalar.activation(
                    gf1_sb[:, nt * H : (nt + 1) * H], sd, AF.Exp, scale=NEG_SLOPE
                )
                if nt in (3, 7, 11):
                    emit_e2rep(nt // 4 + 1)
                psh4 = psh[:, 0:D].rearrange("p (h c) -> p h c", h=H)
                nc.scalar.activation(hp4[:, nt, :, 0:HD], psh4[:], AF.Copy)

            # ---- main: P^T tiles (3-engine split) + aggregation matmuls ----
            def emit_sot(h, half):
                soT = ostagep.tile(
                    [HD + 1, 512], BF16, tag="soT", name=f"soT_{h}_{half}"
                )
                nc.scalar.activation(soT[:], psoT[h * 2 + half][:], AF.Copy)
                nc.sync.dma_start(outs[h * 2 + half], soT[:])

            psoT = {}
            for h in range(H):
                e2h = e2rep[:, h * NI : (h + 1) * NI]
                ptqs = []
                for base in range(0, NT, 4):
                    vq = vqp.tile([P, 4 * NI], BF16, tag="vq")
                    for i in range(4):
                        jt = base + i
                        nc.vector.tensor_scalar(
                            vq[:, i * NI : (i + 1) * NI],
                            e2h,
                            gf1_sb[:, jt * H + h : jt * H + h + 1],
                            f1_sb[:, jt * H + h : jt * H + h + 1],
                            ALU.mult,
                            ALU.max,
                        )
                    ptq = ptqp.tile([P, 4 * NI], BF16, tag="ptq", name=f"ptq_{h}_{base}")
                    nc.vector.tensor_tensor(
                        ptq[:], vq[:], adjt_all[:, base * NI : (base + 4) * NI], ALU.mult
                    )
                    ptqs.append(ptq)
                for hh in (h * 2, h * 2 + 1):
                    psoT[hh] = ps_tile([HD + 1, 512], f"psoT_{hh}", bank=hh)
                for half in range(2):
                    for jt in range(NT):
                        buf = ptqs[jt // 4]
                        i = jt % 4
                        pe(nc.tensor.matmul(
                            psoT[h * 2 + half][:],
                            hplus[:, jt * HP + h * (HD + 1) : jt * HP + (h + 1) * (HD + 1)],
                            buf[:, i * NI + half * 512 : i * NI + half * 512 + 512],
                            start=(jt == 0),
                            stop=(jt == NT - 1),
                            skip_group_check=True,
                        ))
                    emit_sot(h, half)

    if split_waits:
        _split_waits(nc)
    nc.finalize()
    return nc


_NC_CACHE = None


def _get_nc():
    global _NC_CACHE
    if _NC_CACHE is None:
        _NC_CACHE = build_nc()
    return _NC_CACHE


def make_in_maps(x, adj, W, a_src, a_dst):
    x = np.ascontiguousarray(x, dtype=np.float32)
    W = np.ascontiguousarray(W, dtype=np.float32)
    a_src = np.ascontiguousarray(a_src, dtype=np.float32)
    a_dst = np.ascontiguousarray(a_dst, dtype=np.float32)

    A_src = np.zeros((D, H), np.float32)
    A_dst = np.zeros((D, H), np.float32)
    for h in range(H):
        A_src[h * HD : (h + 1) * HD, h] = a_src[h]
        A_dst[h * HD : (h + 1) * HD, h] = a_dst[h]
    Wt = W.T.astype(np.float32)
    wta = np.ascontiguousarray(
        np.concatenate([Wt, Wt @ A_src, Wt @ A_dst], axis=1), dtype=np.float32
    )

    selm = np.zeros((H, H * P), ml_dtypes.bfloat16)
    for h in range(H):
        selm[h, h * P : (h + 1) * P] = 1.0

    in_maps = []
    adjT_cache = {}
    for c in range(NCORES):
        b, ihalf = c // 2, c % 2
        ilo = ihalf * NI
        if b not in adjT_cache:
            adjT_cache[b] = adj[b].astype(ml_dtypes.bfloat16).T
        # column/row permutation: the core's own i-half comes first so the
        # SPMD program can treat block 0 as "own columns" on every core.
        if ihalf == 0:
            xbt_in = x[b].T
            adjt_in = adjT_cache[b][:, ilo : ilo + NI]
        else:
            xbt_in = np.roll(x[b].T, NI, axis=1)
            adjt_in = np.roll(adjT_cache[b], NI, axis=0)[:, ilo : ilo + NI]
        in_maps.append(
            {
                "xbt": np.ascontiguousarray(xbt_in, dtype=np.float16),
                "wta": wta.astype(np.float16),
                "adjtb": np.ascontiguousarray(adjt_in),
                "selm": selm,
            }
        )
    return in_maps


def kernel(x, adj, W, a_src, a_dst):
    in_maps = make_in_maps(x, adj, W, a_src, a_dst)
    nc = _get_nc()
    res = run_bass_kernel_spmd(nc, in_maps, list(range(NCORES)))

    out = np.empty((B, N, D), np.float32)
    for c in range(NCORES):
        b, ihalf = c // 2, c % 2
        ilo = ihalf * NI
        o = np.asarray(res.results[c]["outs"], np.float32)  # [8, 65, 512]
        for h in range(H):
            for half in range(2):
                blk = o[h * 2 + half]
                quot = blk[0:HD, :] / blk[HD, :][None, :]
                out[
                    b,
                    ilo + half * 512 : ilo + (half + 1) * 512,
                    h * HD : (h + 1) * HD,
                ] = quot.T
    return out


# revision 20
# speedup vs baseline: 5.4759x; 1.0028x over previous
"""GAT layer (B=4, N=2048, D=256, H=4) on 8 trn2 NeuronCores.

Sharding: core c -> (b = c//2, i-half = c%2).  Each core computes
out[b, ihalf*1024:(ihalf+1)*1024, :]; h is computed on-device from the full
x[b] (passed pre-transposed as x[b].T, column-permuted so the core's own
i-half comes first).

Math: with z = s_src[i] + s_dst[j], the reference computes
    alpha = softmax_j(mask(leaky_relu(z)));  out = alpha @ h_head.
Softmax is shift-invariant per destination row i, so we use shifted
unnormalized weights (exact same alpha).  F1 = exp(s_dst) is folded into
the attention weights (not into h):
    P[j,i] = adj[j,i] * F1[j] * max(E2[i]*G[j], 1)
           = adj[j,i] * max(E2[i]*GF1[j], F1[j])
with GF1 = exp(0.2*s_dst), E2 = exp(-0.8*s_src)
(using exp(max(a,b)) = max(exp a, exp b) and leaky = max(z, 0.2 z)).
Row sums come from an appended ones-column in the aggregation matmul
stationary: psoT = [h_head | 1].T @ P^T; numerator and denominator are
DMA'd out raw and the final divide + transpose happens on the host.

The per-(head, j-tile) elementwise work P^T is split across three engines:
  D  tiles: DVE tensor_scalar (max(e2rep*gf1, f1)) + tensor_tensor (*adjT)
  S1 tiles: scalar ACT t=Relu(gf1*e2rep - f1); DVE STT pt=(t+f1)*adjT
  S2 tiles: scalar ACT as above; gpsimd STT
"""

import sys

for _p in ("/opt/trn_rl_repo", "/root/.axon_site/_ro/trn_rl_repo"):
    if _p not in sys.path:
        sys.path.insert(0, _p)

import ml_dtypes
import numpy as np

import concourse.bass as bass
import concourse.mybir as mybir
from concourse import tile
from concourse.bass_utils import run_bass_kernel_spmd
from concourse.vector_clock import ScopedClock

F32 = mybir.dt.float32
F16 = mybir.dt.float16
BF16 = mybir.dt.bfloat16
AF = mybir.ActivationFunctionType
ALU = mybir.AluOpType

B, N, D, H, HD = 4, 2048, 256, 4, 64
NEG_SLOPE = 0.2
P = 128
NI = N // 2          # i-rows per core (1024)
NT = N // P          # 16 j tiles
KT = D // P          # 2 k tiles
NCORES = 8
WC = D + 2 * H       # 264 aug cols: [W.T | Wt@A_src | Wt@A_dst]
HP = H * (HD + 1)    # 260 hplus cols per j-tile

# All elementwise tiles run on the DVE: per-tile tensor_scalar (2x mode)
# plus one batched tensor_tensor mask-multiply per run of 4 j-tiles.


def _patch_tile_drain():
    """walrus rejects >1 sem wait on one instruction in this toolchain; split
    the TileContext tail drain's waits across consecutive SP drains."""
    if getattr(tile.TileContext, "_drain_patched", False):
        return

    def _drain_and_barrier(self, tick_clock, wait_clock):
        nc = self.nc
        drain_inst = nc.sync.drain()
        wait_clock.add_sem_waits(
            drain_inst.ins, ScopedClock({None: tick_clock.global_clock})
        )
        si = drain_inst.ins.sync_info
        waits = list(si.on_wait) if (si and si.on_wait) else []
        if len(waits) > 1:
            ups = list(si.on_update) if (si and si.on_update) else []
            drain_inst.ins.sync_info = mybir.SyncInfo(on_wait=waits[:1], on_update=ups)
            for i in range(1, len(waits)):
                extra = nc.sync.drain()
                extra.ins.sync_info = mybir.SyncInfo(
                    on_wait=waits[i : i + 1], on_update=[]
                )
        nc.all_engine_barrier()
        assert self.sems is not None
        popped = nc._tile_sem_poison_stack.pop()
        assert popped is self._sem_poison
        nc.clear_and_free_semaphores(list(self.sems.allocated().values()))
        nc.all_engine_barrier()

    tile.TileContext._drain_and_barrier = _drain_and_barrier
    tile.TileContext._drain_patched = True


def _split_waits(nc, maxw=1):
    """Hoist excess sem waits onto same-engine EventSemaphore carriers placed
    just before the instruction (same engine + program order => equivalent)."""
    n_split = 0
    for f in nc.m.functions:
        for bb in f.blocks:
            insts = list(bb.instructions)
            out = []
            changed = False
            for inst in insts:
                si = inst.sync_info
                waits = list(si.on_wait) if (si and si.on_wait) else []
                if len(waits) > maxw and inst.engine is not None:
                    changed = True
                    extra, keep = waits[:-maxw], waits[-maxw:]
                    for k in range(0, len(extra), maxw):
                        d = mybir.InstEventSemaphore(
                            name=f"{inst.name}-wsplit{k}", ins=[], outs=[]
                        )
                        d.engine = inst.engine
                        d.sync_info = mybir.SyncInfo(
                            on_wait=extra[k : k + maxw], on_update=[]
                        )
                        out.append(d)
                        n_split += 1
                    ups = list(si.on_update) if (si and si.on_update) else []
                    inst.sync_info = mybir.SyncInfo(on_wait=keep, on_update=ups)
                out.append(inst)
            if changed:
                bb.instructions = out
    return n_split


def build_nc(split_waits=True):
    _patch_tile_drain()
    nc = bass.Bass("TRN2", target_bir_lowering=False, debug=False)

    xbt = nc.dram_tensor("xbt", [D, N], F16, kind="ExternalInput")    # x[b].T perm
    wta = nc.dram_tensor("wta", [D, WC], F16, kind="ExternalInput")
    adjtb = nc.dram_tensor("adjtb", [N, NI], BF16, kind="ExternalInput")
    selm = nc.dram_tensor("selm", [H, H * P], BF16, kind="ExternalInput")
    outs = nc.dram_tensor("outs", [2 * H, HD + 1, 512], BF16, kind="ExternalOutput")

    with tile.TileContext(nc) as tc:
        with (
            tc.tile_pool(name="const", bufs=1) as constp,
            tc.tile_pool(name="big", bufs=1) as bigp,
            tc.tile_pool(name="rows", bufs=1) as rowsp,
            tc.tile_pool(name="vqwork", bufs=3) as vqp,
            tc.tile_pool(name="ptq", bufs=6) as ptqp,
            tc.tile_pool(name="ostage", bufs=3) as ostagep,
            tc.tile_pool(name="psall", bufs=1, space="PSUM") as psall,
        ):
            def ps_tile(shape, name, bank):
                return psall.tile(shape, F32, tag=f"bank{bank}", name=name)

            pe_prev = [None]

            def pe(bi):
                # pin PE stream order: PSUM accumulation groups must stay
                # contiguous on PE (interleaving corrupts accumulation on HW)
                if pe_prev[0] is not None:
                    tile.add_dep_helper(bi.ins, pe_prev[0], reason="pe-order")
                pe_prev[0] = bi.ins
                return bi

            # ---- constants ----
            wta_sb = [
                constp.tile([P, WC], F16, tag=f"wta{kt}", name=f"wta_sb{kt}")
                for kt in range(KT)
            ]
            for kt in range(KT):
                nc.sync.dma_start(wta_sb[kt][:], wta[kt * P : (kt + 1) * P, :])
            sel_sb = constp.tile([H, H * P], BF16, tag="selm")
            nc.sync.dma_start(sel_sb[:], selm[:])
            sels = [sel_sb[:, h * P : (h + 1) * P] for h in range(H)]
            wta_r = [wta_sb[kt][:] for kt in range(KT)]

            # ---- big SBUF tensors ----
            xt_raw = bigp.tile([P, KT * N], F16, tag="xtraw")
            xt_r = xt_raw[:]
            adjt_all = bigp.tile([P, NT * NI], BF16, tag="adjt")
            e2rep = bigp.tile([P, H * NI], BF16, tag="e2rep")
            hplus = bigp.tile([P, NT * HP], BF16, tag="hplus")
            f1_sb = bigp.tile([P, NT * H], F32, tag="f1")
            gf1_sb = bigp.tile([P, NT * H], F32, tag="gf1")
            er4 = rowsp.tile([H, NI], BF16, tag="er4")

            nc.gpsimd.memset(hplus[:], 1.0)

            # ---- input DMAs (batched; own-half x first, split so the
            # s_src -> er4 -> e2rep chain unblocks as early as possible) ----
            xbt3 = xbt[:].rearrange("(k p) n -> p k n", p=P)
            xt3 = xt_raw[:].rearrange("p (k n) -> p k n", k=KT)
            adj3_in = adjtb[:].rearrange("(t p) i -> p t i", p=P)
            adj3_sb = adjt_all[:].rearrange("p (t i) -> p t i", t=NT)
            nc.sync.dma_start(xt3[:, :, 0:512], xbt3[:, :, 0:512])
            nc.sync.dma_start(xt3[:, :, 512:NI], xbt3[:, :, 512:NI])
            nc.sync.dma_start(adj3_sb[:, 0:4, :], adj3_in[:, 0:4, :])
            nc.sync.dma_start(adj3_sb[:, 4:8, :], adj3_in[:, 4:8, :])
            nc.sync.dma_start(xt3[:, :, NI:N], xbt3[:, :, NI:N])
            nc.sync.dma_start(adj3_sb[:, 8:12, :], adj3_in[:, 8:12, :])
            nc.sync.dma_start(adj3_sb[:, 12:16, :], adj3_in[:, 12:16, :])

            # ---- s_srcT (all heads) -> E2 rows [4, NI] ----
            def emit_e2rep_chunk(h, c):
                psb = ps_tile([P, 512], f"psb_{h}_{c}", bank=7)
                pe(nc.tensor.matmul(
                    psb[:], sels[h], er4[0:H, c * 512 : (c + 1) * 512]
                ))
                nc.scalar.activation(
                    e2rep[:, h * NI + c * 512 : h * NI + (c + 1) * 512],
                    psb[:],
                    AF.Copy,
                )

            for c in range(2):
                pss = ps_tile([H, 512], f"pss_{c}", bank=c)
                for kt in range(KT):
                    pe(nc.tensor.matmul(
                        pss[:],
                        wta_r[kt][:, D : D + H],
                        xt_r[:, kt * N + c * 512 : kt * N + (c + 1) * 512],
                        start=(kt == 0),
                        stop=(kt == KT - 1),
                    ))
                nc.scalar.activation(
                    er4[:, c * 512 : (c + 1) * 512],
                    pss[:],
                    AF.Exp,
                    scale=-(1.0 - NEG_SLOPE),
                )
                emit_e2rep_chunk(0, c)

            # ---- e2rep: broadcast E2 across partitions via a selector
            # matmul on PSUM bank 7 + scalar copy; h1..h3 are emitted from
            # inside the nt loop to interleave with the psh stream ----
            def emit_e2rep(h):
                for c in range(2):
                    emit_e2rep_chunk(h, c)

            # ---- h phase: psh = x @ wta ; f1/gf1/nf1 ; hplus (bf16+ones) ----
            hp4 = hplus[:].rearrange("p (t h c) -> p t h c", t=NT, h=H)
            for nt in range(NT):
                psh = ps_tile([P, WC], f"psh_{nt}", bank=4 + nt % 3)
                for kt in range(KT):
                    pe(nc.tensor.matmul(
                        psh[:],
                        xt_r[:, kt * N + nt * P : kt * N + (nt + 1) * P],
                        wta_r[kt][:],
                        start=(kt == 0),
                        stop=(kt == KT - 1),
                    ))
                sd = psh[:, D + H : D + 2 * H]
                nc.scalar.activation(
                    f1_sb[:, nt * H : (nt + 1) * H], sd, AF.Exp
                )
                nc.scalar.activation(
                    gf1_sb[:, nt * H : (nt + 1) * H], sd, AF.Exp, scale=NEG_SLOPE
                )
                if nt in (3, 7, 11):
                    emit_e2rep(nt // 4 + 1)
                psh4 = psh[:, 0:D].rearrange("p (h c) -> p h c", h=H)
                nc.scalar.activation(hp4[:, nt, :, 0:HD], psh4[:], AF.Copy)

            # ---- main: P^T tiles (3-engine split) + aggregation matmuls ----
            def emit_sot(h, half):
                soT = ostagep.tile(
                    [HD + 1, 512], BF16, tag="soT", name=f"soT_{h}_{half}"
                )
                nc.scalar.activation(soT[:], psoT[h * 2 + half][:], AF.Copy)
                nc.sync.dma_start(outs[h * 2 + half], soT[:])

            psoT = {}
            for h in range(H):
                e2h = e2rep[:, h * NI : (h + 1) * NI]
                ptqs = []
                for base in range(0, NT, 4):
                    vq = vqp.tile([P, 4 * NI], BF16, tag="vq")
                    for i in range(4):
                        jt = base + i
                        nc.vector.tensor_scalar(
                            vq[:, i * NI : (i + 1) * NI],
                            e2h,
                            gf1_sb[:, jt * H + h : jt * H + h + 1],
                            f1_sb[:, jt * H + h : jt * H + h + 1],
                            ALU.mult,
                            ALU.max,
                        )
                    ptq = ptqp.tile([P, 4 * NI], BF16, tag="ptq", name=f"ptq_{h}_{base}")
                    nc.vector.tensor_tensor(
                        ptq[:], vq[:], adjt_all[:, base * NI : (base + 4) * NI], ALU.mult
                    )
                    ptqs.append(ptq)
                for hh in (h * 2, h * 2 + 1):
                    psoT[hh] = ps_tile([HD + 1, 512], f"psoT_{hh}", bank=hh)
                for half in range(2):
                    for jt in range(NT):
                        buf = ptqs[jt // 4]
                        i = jt % 4
                        pe(nc.tensor.matmul(
                            psoT[h * 2 + half][:],
                            hplus[:, jt * HP + h * (HD + 1) : jt * HP + (h + 1) * (HD + 1)],
                            buf[:, i * NI + half * 512 : i * NI + half * 512 + 512],
                            start=(jt == 0),
                            stop=(jt == NT - 1),
                            skip_group_check=True,
                        ))
                    emit_sot(h, half)

    if split_waits:
        _split_waits(nc)
    nc.finalize()
    return nc


_NC_CACHE = None


def _get_nc():
    global _NC_CACHE
    if _NC_CACHE is None:
        _NC_CACHE = build_nc()
    return _NC_CACHE


def make_in_maps(x, adj, W, a_src, a_dst):
    x = np.ascontiguousarray(x, dtype=np.float32)
    W = np.ascontiguousarray(W, dtype=np.float32)
    a_src = np.ascontiguousarray(a_src, dtype=np.float32)
    a_dst = np.ascontiguousarray(a_dst, dtype=np.float32)

    A_src = np.zeros((D, H), np.float32)
    A_dst = np.zeros((D, H), np.float32)
    for h in range(H):
        A_src[h * HD : (h + 1) * HD, h] = a_src[h]
        A_dst[h * HD : (h + 1) * HD, h] = a_dst[h]
    Wt = W.T.astype(np.float32)
    wta = np.ascontiguousarray(
        np.concatenate([Wt, Wt @ A_src, Wt @ A_dst], axis=1), dtype=np.float32
    )

    selm = np.zeros((H, H * P), ml_dtypes.bfloat16)
    for h in range(H):
        selm[h, h * P : (h + 1) * P] = 1.0

    in_maps = []
    adjT_cache = {}
    for c in range(NCORES):
        b, ihalf = c // 2, c % 2
        ilo = ihalf * NI
        if b not in adjT_cache:
            adjT_cache[b] = adj[b].astype(ml_dtypes.bfloat16).T
        # column/row permutation: the core's own i-half comes first so the
        # SPMD program can treat block 0 as "own columns" on every core.
        if ihalf == 0:
            xbt_in = x[b].T
            adjt_in = adjT_cache[b][:, ilo : ilo + NI]
        else:
            xbt_in = np.roll(x[b].T, NI, axis=1)
            adjt_in = np.roll(adjT_cache[b], NI, axis=0)[:, ilo : ilo + NI]
        in_maps.append(
            {
                "xbt": np.ascontiguousarray(xbt_in, dtype=np.float16),
                "wta": wta.astype(np.float16),
                "adjtb": np.ascontiguousarray(adjt_in),
                "selm": selm,
            }
        )
    return in_maps


def kernel(x, adj, W, a_src, a_dst):
    in_maps = make_in_maps(x, adj, W, a_src, a_dst)
    nc = _get_nc()
    res = run_bass_kernel_spmd(nc, in_maps, list(range(NCORES)))

    out = np.empty((B, N, D), np.float32)
    for c in range(NCORES):
        b, ihalf = c // 2, c % 2
        ilo = ihalf * NI
        o = np.asarray(res.results[c]["outs"], np.float32)  # [8, 65, 512]
        for h in range(H):
            for half in range(2):
                blk = o[h * 2 + half]
                quot = blk[0:HD, :] / blk[HD, :][None, :]
                out[
                    b,
                    ilo + half * 512 : ilo + (half + 1) * 512,
                    h * HD : (h + 1) * HD,
                ] = quot.T
    return out
